# revision 24
# baseline (speedup 1.0000x reference)
"""Trainium2 Bass kernel for nn_NodeEncoder (2-layer SAGEConv GNN).

Self-contained: takes FULL inputs, shards receivers across 8 NeuronCores,
runs a Bass/Tile kernel via run_bass_kernel_spmd, returns the FULL output.

Math per layer (SAGEConv, degree_norm=True, self loops):
  x_upd[r] = sum_{e: recv=r} w_e * x[s_e] + selfw_r * x[r],
     w_e = (ds[s_e] * dr[r_e]^3)^-1/2, selfw_n = (ds[n]*dr[n]^3)^-1/2
  out = x @ Wa + x_upd @ Wb + b   (+relu after layer 1)

Device mapping (per core, receivers sharded, 98 windows of 128 nodes):
  - psum_agg[f, r]  = sum_chunks  msg_chunk[e,f].T @ onehot_chunk[e,r]
      onehot[e, r] = w_e * (recv_e == r)  -- HOST-precomputed, DMA streamed
      L1 msgs: HOST-pregathered sequential stream;  L2: dma_gather from
      the AllGathered h1 table on async SWDGE queues 1-3
      self loop: lhsT = row tile [n,f], rhs = diag(selfw) (in onehot stream)
  - psum_xt[f, n]   = rowtile[n,f].T @ I       (transpose via PE)
  - psum_out[n, fo] = U0.T@Bmat (bias) + xt[f,n].T@Wa + summed[f,n].T@Wb
  - node tables (x0, h1, out) use a permuted "group" layout so loads and
    stores of 8-window groups are single contiguous DMAs
"""

import numpy as np
import ml_dtypes

BF16 = ml_dtypes.bfloat16
N = 100000
E = 600000
D = 128
NC = 8
P = 128
SLICE = N // NC                 # 12500
NW = (SLICE + P - 1) // P       # 98 windows
SLICE_PAD = NW * P              # 12544
GRPW = 8                        # windows per group
NG = (NW + GRPW - 1) // GRPW    # 13 groups (12x8 + 1x2)
NPAD = SLICE_PAD * NC           # 100352
NBANKS = 4
BROWS = NPAD // NBANKS          # 25088 (< 32768, int16-safe)
GATHER_BATCH = 2048             # idxs per dma_gather instruction
PAD_NEG = False                 # -1 idx pads (skipped) vs safe 0 pads
LCH = 24                        # stream-load piece size (chunks)

_last_results = None


def _grp_sizes():
    return [min(GRPW, NW - g * GRPW) for g in range(NG)]


def _perm():
    """prow[l] = permuted row of local node l (group layout)."""
    l = np.arange(SLICE_PAD)
    k = l // P
    p = l % P
    g = k // GRPW
    j = k - g * GRPW
    base = np.minimum(g, 12) * (GRPW * P) * 1  # recompute below correctly
    # base offset of group g = sum of sizes of previous groups * P
    sizes = _grp_sizes()
    starts = np.concatenate([[0], np.cumsum([s * P for s in sizes])])
    nwg = np.array(sizes)[g]
    return starts[g] + p * nwg + j


def _build_program(chunks1, chunks2, nchunks_b, chunk_of2,
                   idxcols, tot1, totoh2):
    import concourse.bacc as bacc
    import concourse.mybir as mybir
    import concourse.tile as tile
    from concourse.masks import make_identity

    DT = mybir.dt.float32
    DT2 = mybir.dt.bfloat16
    sizes = _grp_sizes()
    gstart = np.concatenate([[0], np.cumsum([s * P for s in sizes])])

    # window -> (oh-chunk offset, n edge chunks) per layer
    t1 = np.concatenate([[0], np.cumsum(chunks1)])
    uk = chunks2.sum(axis=1)
    ot2 = np.concatenate([[0], np.cumsum(uk + 1)])

    nc = bacc.Bacc("TRN2", target_bir_lowering=False, num_swdge_queues=4)

    x0g = nc.dram_tensor("x0g", [SLICE_PAD, D], DT2, kind="ExternalInput")
    msg1 = nc.dram_tensor("msg1", [P, max(tot1, 1), D], DT2, kind="ExternalInput")
    recv1 = nc.dram_tensor("recv1", [P, max(tot1, 1)], DT, kind="ExternalInput")
    wch1 = nc.dram_tensor("wch1", [P, max(tot1, 1)], DT, kind="ExternalInput")
    selfw1 = nc.dram_tensor("selfw1", [P, NW], DT, kind="ExternalInput")
    oh2 = nc.dram_tensor("oh2", [P, totoh2, D], DT2, kind="ExternalInput")
    gidx = nc.dram_tensor("gidx", [P, idxcols], mybir.dt.int16, kind="ExternalInput")
    wa1 = nc.dram_tensor("wa1", [D, D], DT2, kind="ExternalInput")
    wb1 = nc.dram_tensor("wb1", [D, D], DT2, kind="ExternalInput")
    wa2 = nc.dram_tensor("wa2", [D, D], DT2, kind="ExternalInput")
    wb2 = nc.dram_tensor("wb2", [D, D], DT2, kind="ExternalInput")
    bm1 = nc.dram_tensor("bm1", [D, D], DT2, kind="ExternalInput")
    bm2 = nc.dram_tensor("bm2", [D, D], DT2, kind="ExternalInput")
    h1s = nc.dram_tensor("h1s", [SLICE_PAD, D], DT2)
    h1f = [nc.dram_tensor(f"h1f{b}", [BROWS, D], DT2, addr_space="Shared")
           for b in range(NBANKS)]
    out = nc.dram_tensor("out", [SLICE_PAD, D], DT2, kind="ExternalOutput")

    bank_col0 = np.concatenate([[0], np.cumsum(nchunks_b * P // 16)]).astype(int)

    # 2048-idx gather batches: per bank, runs of <= 16 chunks
    batches = []
    for b in range(NBANKS):
        c0 = 0
        while c0 < nchunks_b[b]:
            nb_ = min(GATHER_BATCH // P, int(nchunks_b[b]) - c0)
            batches.append((b, c0, nb_))
            c0 += nb_
    pos2batch = {}
    for bi, (b, c0, nchk) in enumerate(batches):
        for jj in range(nchk):
            pos2batch[(b, c0 + jj)] = (bi, jj)
    c2b = {}
    for k in range(NW):
        for b in range(NBANKS):
            for t in range(chunks2[k, b]):
                c2b[(k, b, t)] = pos2batch[(b, chunk_of2[k, b] + t)]
    bank_batches = {b: [bi for bi, (bb, _, _) in enumerate(batches) if bb == b]
                    for b in range(NBANKS)}

    with tile.TileContext(nc) as tc:
        with tc.tile_pool(name="const", bufs=1) as cpool, \
             tc.tile_pool(name="meta", bufs=1) as mpool, \
             tc.tile_pool(name="ms", bufs=2) as mspool, \
             tc.tile_pool(name="oh", bufs=2) as ohpool, \
             tc.tile_pool(name="ohd", bufs=6) as ohdpool, \
             tc.tile_pool(name="gat", bufs=4) as gpool, \
             tc.tile_pool(name="grp", bufs=2) as grpool, \
             tc.tile_pool(name="sm", bufs=3) as smpool, \
             tc.tile_pool(name="st", bufs=2) as stpool, \
             tc.tile_pool(name="pa", bufs=2, space="PSUM") as papool, \
             tc.tile_pool(name="px", bufs=2, space="PSUM") as pxpool, \
             tc.tile_pool(name="po", bufs=2, space="PSUM") as popool:

            ident_f = cpool.tile([P, P], DT)
            make_identity(nc, ident_f[:])
            ident = cpool.tile([P, P], DT2)
            nc.vector.tensor_copy(ident[:], ident_f[:])
            iota_i = cpool.tile([P, P], mybir.dt.int32)
            nc.gpsimd.iota(iota_i[:], pattern=[[1, P]], base=0, channel_multiplier=0)
            iota_f = cpool.tile([P, P], DT2)
            nc.vector.tensor_copy(iota_f[:], iota_i[:])
            # U0: row 0 all ones (for bias matmul)
            u0 = cpool.tile([P, P], DT2)
            nc.vector.memset(u0[:], 0.0)
            nc.vector.memset(u0[0:1, :], 1.0)

            wa = [cpool.tile([P, D], DT2, name=f"wa{l}") for l in range(2)]
            wb = [cpool.tile([P, D], DT2, name=f"wb{l}") for l in range(2)]
            bm = [cpool.tile([P, D], DT2, name=f"bm{l}") for l in range(2)]
            for li, (wat, wbt, bmt) in enumerate(((wa1, wb1, bm1), (wa2, wb2, bm2))):
                nc.sync.dma_start(out=wa[li][:], in_=wat[:, :])
                nc.sync.dma_start(out=wb[li][:], in_=wbt[:, :])
                nc.sync.dma_start(out=bm[li][:], in_=bmt[:, :])

            gidx_sb = mpool.tile([P, idxcols], mybir.dt.int16)
            nc.sync.dma_start(out=gidx_sb[:], in_=gidx[:])
            recv1_sb = mpool.tile([P, max(tot1, 1)], DT)
            nc.sync.dma_start(out=recv1_sb[:], in_=recv1[:])
            wch1_sb = mpool.tile([P, max(tot1, 1)], DT)
            nc.sync.dma_start(out=wch1_sb[:], in_=wch1[:])
            selfw_sb = mpool.tile([P, NW], DT)
            nc.sync.dma_start(out=selfw_sb[:], in_=selfw1[:])

            relu_t = mybir.ActivationFunctionType.Relu
            copy_t = mybir.ActivationFunctionType.Copy

            for layer in range(2):
                xsrc = x0g if layer == 0 else h1s
                ot = ot2
                dst = h1s if layer == 0 else out

                # --- L2: lazy 2048-idx gather batches on async queues 1-3 ---
                gtiles = {}
                bank_next = [0] * NBANKS

                def ensure_batch(bi):
                    b = batches[bi][0]
                    while bi not in gtiles:
                        nb = bank_batches[b][bank_next[b]]
                        bank_next[b] += 1
                        _, c0, nchk = batches[nb]
                        nidx = nchk * P
                        gt = gpool.tile([P, nchk, D], DT2,
                                        tag=f"gq{nb % 3}")
                        col0 = bank_col0[b] + c0 * P // 16
                        nc.gpsimd.dma_gather(
                            gt[:],
                            h1f[b][:, :],
                            gidx_sb[:, col0:col0 + nidx // 16],
                            nidx, nidx, D,
                            single_packet=False,
                            queue_num=1 + (nb % 3),
                        )
                        gtiles[nb] = gt
                    return gtiles[bi]

                # --- stream state: msg pieces (L1) and oh pieces (L2) ---
                oh_piece = [None, -1, -1]   # tile, t0, t1
                ms_piece = [None, -1, -1]

                def oh_slice(t):
                    if not (oh_piece[1] <= t < oh_piece[2]):
                        t0 = t
                        te = min(t0 + LCH, totoh2)
                        pt = ohpool.tile([P, te - t0, D], DT2, tag="ohp")
                        nc.sync.dma_start(out=pt[:], in_=oh2[:, t0:te, :])
                        oh_piece[0], oh_piece[1], oh_piece[2] = pt, t0, te
                    return oh_piece[0][:, t - oh_piece[1], :]

                def ms_slice(t):
                    if not (ms_piece[1] <= t < ms_piece[2]):
                        t0 = t
                        te = min(t0 + LCH, tot1)
                        pt = mspool.tile([P, te - t0, D], DT2, tag="msp")
                        nc.sync.dma_start(out=pt[:], in_=msg1[:, t0:te, :])
                        ms_piece[0], ms_piece[1], ms_piece[2] = pt, t0, te
                    return ms_piece[0][:, t - ms_piece[1], :]

                for g in range(NG):
                    nwg = sizes[g]
                    grp = grpool.tile([P, nwg, D], DT2, tag="grp")
                    nc.sync.dma_start(
                        out=grp[:], in_=xsrc[gstart[g]:gstart[g] + nwg * P, :])
                    stage = stpool.tile([P, nwg, D], DT2, tag=f"stg{layer}")

                    for j in range(nwg):
                        k = g * GRPW + j
                        psum = papool.tile([P, P], mybir.dt.float32, space="PSUM")
                        first = True
                        if layer == 0:
                            for t in range(chunks1[k]):
                                mt = ms_slice(t1[k] + t)
                                oht = ohdpool.tile([P, P], DT2, tag="ohb")
                                nc.vector.tensor_scalar(
                                    out=oht[:], in0=iota_f[:],
                                    scalar1=recv1_sb[:, t1[k] + t:t1[k] + t + 1],
                                    scalar2=wch1_sb[:, t1[k] + t:t1[k] + t + 1],
                                    op0=mybir.AluOpType.is_equal,
                                    op1=mybir.AluOpType.mult)
                                nc.tensor.matmul(out=psum[:], lhsT=mt, rhs=oht[:],
                                                 start=first, stop=False)
                                first = False
                            # self: diag(selfw) built from identity
                            oht = ohdpool.tile([P, P], DT2, tag="ohb")
                            nc.vector.tensor_scalar(
                                out=oht[:], in0=ident[:],
                                scalar1=selfw_sb[:, k:k + 1], scalar2=None,
                                op0=mybir.AluOpType.mult)
                            nc.tensor.matmul(out=psum[:], lhsT=grp[:, j, :],
                                             rhs=oht[:], start=first, stop=True)
                        else:
                            u = 0
                            for b in range(NBANKS):
                                for t in range(chunks2[k, b]):
                                    bi, jj = c2b[(k, b, t)]
                                    gt = ensure_batch(bi)
                                    oht = oh_slice(ot[k] + u)
                                    nc.tensor.matmul(
                                        out=psum[:], lhsT=gt[:, jj, :], rhs=oht,
                                        start=first, stop=False)
                                    first = False
                                    u += 1
                            oht = oh_slice(ot[k] + uk[k])
                            nc.tensor.matmul(out=psum[:], lhsT=grp[:, j, :],
                                             rhs=oht, start=first, stop=True)

                        psx = pxpool.tile([P, P], mybir.dt.float32, space="PSUM")
                        nc.tensor.matmul(out=psx[:], lhsT=grp[:, j, :], rhs=ident[:],
                                         start=True, stop=True)

                        summed = smpool.tile([P, P], DT2, tag="summed")
                        nc.vector.tensor_copy(summed[:], psum[:])
                        xt = smpool.tile([P, P], DT2, tag="xt")
                        nc.vector.tensor_copy(xt[:], psx[:])

                        pso = popool.tile([P, P], mybir.dt.float32, space="PSUM")
                        nc.tensor.matmul(out=pso[:], lhsT=u0[:], rhs=bm[layer][:],
                                         start=True, stop=False)
                        nc.tensor.matmul(out=pso[:], lhsT=xt[:], rhs=wa[layer][:],
                                         start=False, stop=False)
                        nc.tensor.matmul(out=pso[:], lhsT=summed[:], rhs=wb[layer][:],
                                         start=False, stop=True)
                        nc.scalar.activation(
                            out=stage[:, j, :], in_=pso[:],
                            func=relu_t if layer == 0 else copy_t)

                    nc.sync.dma_start(
                        out=dst[gstart[g]:gstart[g] + nwg * P, :], in_=stage[:])

                if layer == 0:
                    rng_rows = SLICE_PAD // NBANKS
                    for b in range(NBANKS):
                        nc.gpsimd.collective_compute(
                            kind="AllGather",
                            op=mybir.AluOpType.bypass,
                            replica_groups=[list(range(NC))],
                            ins=[h1s[b * rng_rows:(b + 1) * rng_rows, :]],
                            outs=[h1f[b][:, :]],
                        )
    nc.compile()
    return nc


def _prep(gid, senders, receivers, emb_table, W1, b1, W2, b2):
    """Host-side layout computation. Returns (layout, in_maps)."""
    gid = np.asarray(gid)
    s = np.asarray(senders).astype(np.int64)
    r = np.asarray(receivers).astype(np.int64)
    emb = np.asarray(emb_table, dtype=np.float32)
    W1 = np.asarray(W1, np.float32)
    W2 = np.asarray(W2, np.float32)
    b1 = np.asarray(b1, np.float32)
    b2 = np.asarray(b2, np.float32)

    x0 = emb[gid]
    x_bf = x0.astype(BF16)

    ds = (1 + np.bincount(s, minlength=N)).astype(np.float64)
    dr = (1 + np.bincount(r, minlength=N)).astype(np.float64)
    wch = (1.0 / np.sqrt(ds[s] * dr[r] ** 3)).astype(np.float32)
    selfw = (1.0 / np.sqrt(ds * dr ** 3)).astype(np.float32)

    prow = _perm()                      # local l -> permuted row
    RNG = SLICE_PAD // NBANKS           # 3136 rows per h1s range

    core_of = r // SLICE
    rloc = r % SLICE
    k_all = rloc // P
    rcol_all = rloc % P
    # bank = sender's local permuted-row range; h1f[bank] = AllGather of that
    # range across cores -> row = src_core*RNG + offset
    lam = prow[s % SLICE]
    bank_all = lam // RNG
    brow_all = ((s // SLICE) * RNG + lam % RNG).astype(np.int16)

    # ---- per-core per-window counts -> shared chunk layout ----
    cnt1 = np.zeros((NC, NW), np.int64)
    np.add.at(cnt1, (core_of, k_all), 1)
    chunks1 = np.ceil(cnt1.max(axis=0) / P).astype(np.int64)   # [NW]
    cnt2 = np.zeros((NC, NW, NBANKS), np.int64)
    np.add.at(cnt2, (core_of, k_all, bank_all), 1)
    chunks2 = np.ceil(cnt2.max(axis=0) / P).astype(np.int64)   # [NW, NBANKS]

    tot1 = int(chunks1.sum())
    totoh1 = int((chunks1 + 1).sum())
    uk = chunks2.sum(axis=1)
    totoh2 = int((uk + 1).sum())

    # bank chunk positions: bank b chunks ordered by (k, t)
    chunk_of2 = np.zeros((NW, NBANKS), np.int64)
    nchunks_b = np.zeros(NBANKS, np.int64)
    for b in range(NBANKS):
        pos = 0
        for k in range(NW):
            chunk_of2[k, b] = pos
            pos += chunks2[k, b]
        nchunks_b[b] = pos

    idxcols = int(nchunks_b.sum()) * P // 16

    ot1 = np.concatenate([[0], np.cumsum(chunks1 + 1)])
    t1o = np.concatenate([[0], np.cumsum(chunks1)])
    ot2 = np.concatenate([[0], np.cumsum(uk + 1)])
    # within-window oh position offset for (b): cumsum of chunks2 row
    boff = np.zeros((NW, NBANKS), np.int64)
    for k in range(NW):
        acc = 0
        for b in range(NBANKS):
            boff[k, b] = acc
            acc += chunks2[k, b]

    Wa1 = W1[:D].astype(BF16)
    Wb1 = W1[D:].astype(BF16)
    Wa2 = W2[:D].astype(BF16)
    Wb2 = W2[D:].astype(BF16)
    bm1 = np.zeros((D, D), BF16)
    bm1[0, :] = b1.astype(BF16)
    bm2 = np.zeros((D, D), BF16)
    bm2[0, :] = b2.astype(BF16)

    # selfw values per (window, p) for each core, 0 on pad nodes
    in_maps = []
    for c in range(NC):
        m = core_of == c
        km = k_all[m]
        rcolm = rcol_all[m]
        sm = s[m]
        wchm = wch[m]
        bankm = bank_all[m]
        browm = brow_all[m]

        # ---- layer 1 slots: sort by window ----
        o1 = np.argsort(km, kind="stable")
        k1s = km[o1]
        # within-window running index
        ww = np.ones(len(k1s), np.int64)
        first = np.where(np.diff(k1s, prepend=-1) != 0)[0]
        run = np.arange(len(k1s)) - first[np.searchsorted(first, np.arange(len(k1s)), side="right") - 1]
        t_1 = run // P
        p_1 = run % P
        tglob1 = t1o[k1s] + t_1
        ohglob1 = ot1[k1s] + t_1

        msg1 = np.zeros((P, max(tot1, 1), D), BF16)
        msg1[p_1, tglob1, :] = x_bf[sm[o1]]
        recv1 = np.full((P, max(tot1, 1)), -1000.0, np.float32)
        recv1[p_1, tglob1] = rcolm[o1]
        wch1 = np.ones((P, max(tot1, 1)), np.float32)
        wch1[p_1, tglob1] = wchm[o1]
        # self-loop weights per (p, window), 0 on pad nodes
        node_l = np.arange(SLICE_PAD)
        valid = node_l < SLICE
        kk = node_l // P
        pp = node_l % P
        sw = np.zeros(SLICE_PAD, np.float32)
        sw[valid] = selfw[c * SLICE + node_l[valid]]
        selfw1 = np.zeros((P, NW), np.float32)
        selfw1[pp, kk] = sw

        # ---- layer 2 slots: sort by (window, bank) ----
        o2 = np.lexsort((bankm, km))
        k2s = km[o2]
        b2s = bankm[o2]
        gid2 = k2s * NBANKS + b2s
        first2 = np.where(np.diff(gid2, prepend=-1) != 0)[0]
        run2 = np.arange(len(gid2)) - first2[np.searchsorted(first2, np.arange(len(gid2)), side="right") - 1]
        t_2 = run2 // P
        p_2 = run2 % P
        ohglob2 = ot2[k2s] + boff[k2s, b2s] + t_2
        oh2 = np.zeros((P, totoh2, D), BF16)
        oh2[p_2, ohglob2, rcolm[o2]] = wchm[o2].astype(BF16)
        oh2[pp, ot2[kk] + uk[kk], pp] = sw.astype(BF16)

        # gather idx per bank. Pads at each (k,b)-region tail become -1
        # (skipped by the gather ucode), except the partial trailing
        # 16-group which uses safe idx 0 (killed by zero one-hot cols).
        idx16 = []
        cpos2 = chunk_of2[k2s, b2s] + t_2
        cnt_cb = cnt2[c]                       # [NW, NBANKS] this core's counts
        for b in range(NBANKS):
            mb = b2s == b
            st = np.zeros(int(nchunks_b[b]) * P, np.int16)
            if PAD_NEG:
                for k in range(NW):
                    nchk = chunks2[k, b]
                    if nchk == 0:
                        continue
                    lo = chunk_of2[k, b] * P
                    valid16 = -(-int(cnt_cb[k, b]) // 16) * 16
                    st[lo + valid16: lo + nchk * P] = -1
            st[cpos2[mb] * P + p_2[mb]] = browm[o2][mb]
            a = st.reshape(len(st) // 16, 16).T.copy()
            idx16.append(np.tile(a, (8, 1)))
        gidx_np = np.concatenate(idx16, axis=1) if idxcols else np.zeros((P, 0), np.int16)

        # x0 permuted table
        x0g = np.zeros((SLICE_PAD, D), BF16)
        x0g[prow[node_l[valid]]] = x_bf[c * SLICE + node_l[valid]]

        in_maps.append({
            "x0g": x0g, "msg1": msg1, "recv1": recv1, "wch1": wch1,
            "selfw1": selfw1, "oh2": oh2, "gidx": gidx_np,
            "wa1": Wa1, "wb1": Wb1, "wa2": Wa2, "wb2": Wb2,
            "bm1": bm1, "bm2": bm2,
        })

    layout = dict(chunks1=chunks1, chunks2=chunks2, nchunks_b=nchunks_b,
                  chunk_of2=chunk_of2, idxcols=idxcols,
                  tot1=tot1, totoh2=totoh2)
    return layout, in_maps, prow


def kernel(gid, senders, receivers, is_training, emb_table, W1, b1, W2, b2):
    global _last_results
    from concourse.bass_utils import run_bass_kernel_spmd

    layout, in_maps, prow = _prep(gid, senders, receivers, emb_table,
                                  W1, b1, W2, b2)
    nc = _build_program(layout["chunks1"], layout["chunks2"],
                        layout["nchunks_b"], layout["chunk_of2"],
                        layout["idxcols"], layout["tot1"], layout["totoh2"])

    res = run_bass_kernel_spmd(nc, in_maps, core_ids=list(range(NC)))
    _last_results = res

    out = np.empty((N, D), np.float32)
    l = np.arange(SLICE)
    for c in range(NC):
        out[c * SLICE:(c + 1) * SLICE] = \
            res.results[c]["out"][prow[l]].astype(np.float32)
    return out


# revision 26
# speedup vs baseline: 1.1278x; 1.1278x over previous
"""Trainium2 Bass kernel for nn_NodeEncoder (2-layer SAGEConv GNN).

Self-contained: takes FULL inputs, shards receivers across 8 NeuronCores,
runs a Bass/Tile kernel via run_bass_kernel_spmd, returns the FULL output.

Math per layer (SAGEConv, degree_norm=True, self loops):
  x_upd[r] = sum_{e: recv=r} w_e * x[s_e] + selfw_r * x[r],
     w_e = (ds[s_e] * dr[r_e]^3)^-1/2, selfw_n = (ds[n]*dr[n]^3)^-1/2
  out = x @ Wa + x_upd @ Wb + b   (+relu after layer 1)

Device mapping (per core, receivers sharded, 98 windows of 128 nodes):
  - psum_agg[f, r]  = sum_chunks  msg_chunk[e,f].T @ onehot_chunk[e,r]
      onehot[e, r] = w_e * (recv_e == r)  -- HOST-precomputed, DMA streamed
      L1 msgs: HOST-pregathered sequential stream;  L2: dma_gather from
      the AllGathered h1 table on async SWDGE queues 1-3
      self loop: lhsT = row tile [n,f], rhs = diag(selfw) (in onehot stream)
  - psum_xt[f, n]   = rowtile[n,f].T @ I       (transpose via PE)
  - psum_out[n, fo] = U0.T@Bmat (bias) + xt[f,n].T@Wa + summed[f,n].T@Wb
  - node tables (x0, h1, out) use a permuted "group" layout so loads and
    stores of 8-window groups are single contiguous DMAs
"""

import numpy as np
import ml_dtypes

BF16 = ml_dtypes.bfloat16
N = 100000
E = 600000
D = 128
NC = 8
P = 128
SLICE = N // NC                 # 12500
NW = (SLICE + P - 1) // P       # 98 windows
SLICE_PAD = NW * P              # 12544
GRPW = 8                        # windows per group
NG = (NW + GRPW - 1) // GRPW    # 13 groups (12x8 + 1x2)
NPAD = SLICE_PAD * NC           # 100352
NBANKS = 4
BROWS = NPAD // NBANKS          # 25088 (< 32768, int16-safe)
GATHER_BATCH = 2048             # idxs per dma_gather instruction
PAD_NEG = False                 # -1 idx pads (skipped) vs safe 0 pads
LCH = 24                        # stream-load piece size (chunks)

_last_results = None


def _grp_sizes():
    return [min(GRPW, NW - g * GRPW) for g in range(NG)]


def _perm():
    """prow[l] = permuted row of local node l (group layout)."""
    l = np.arange(SLICE_PAD)
    k = l // P
    p = l % P
    g = k // GRPW
    j = k - g * GRPW
    base = np.minimum(g, 12) * (GRPW * P) * 1  # recompute below correctly
    # base offset of group g = sum of sizes of previous groups * P
    sizes = _grp_sizes()
    starts = np.concatenate([[0], np.cumsum([s * P for s in sizes])])
    nwg = np.array(sizes)[g]
    return starts[g] + p * nwg + j


def _build_program(chunks1, chunks2, nchunks_b, chunk_of2,
                   idxcols, tot1, totoh2, hoff, toth):
    import concourse.bacc as bacc
    import concourse.mybir as mybir
    import concourse.tile as tile
    from concourse.masks import make_identity

    DT = mybir.dt.float32
    DT2 = mybir.dt.bfloat16
    sizes = _grp_sizes()
    gstart = np.concatenate([[0], np.cumsum([s * P for s in sizes])])

    # window -> (oh-chunk offset, n edge chunks) per layer
    t1 = np.concatenate([[0], np.cumsum(chunks1)])
    uk = chunks2.sum(axis=1)
    ot2 = np.concatenate([[0], np.cumsum(uk + 1)])

    nc = bacc.Bacc("TRN2", target_bir_lowering=False, num_swdge_queues=4)

    x0g = nc.dram_tensor("x0g", [SLICE_PAD, D], DT2, kind="ExternalInput")
    msg1 = nc.dram_tensor("msg1", [P, max(tot1, 1), D], DT2, kind="ExternalInput")
    recv1 = nc.dram_tensor("recv1", [P, max(tot1, 1)], DT, kind="ExternalInput")
    wch1 = nc.dram_tensor("wch1", [P, max(tot1, 1)], DT, kind="ExternalInput")
    selfw1 = nc.dram_tensor("selfw1", [P, NW], DT, kind="ExternalInput")
    oh2 = nc.dram_tensor("oh2", [P, totoh2, D], DT2, kind="ExternalInput")
    oh1h = nc.dram_tensor("oh1h", [P, max(toth, 1), D], DT2, kind="ExternalInput")
    gidx = nc.dram_tensor("gidx", [P, idxcols], mybir.dt.int16, kind="ExternalInput")
    wa1 = nc.dram_tensor("wa1", [D, D], DT2, kind="ExternalInput")
    wb1 = nc.dram_tensor("wb1", [D, D], DT2, kind="ExternalInput")
    wa2 = nc.dram_tensor("wa2", [D, D], DT2, kind="ExternalInput")
    wb2 = nc.dram_tensor("wb2", [D, D], DT2, kind="ExternalInput")
    bm1 = nc.dram_tensor("bm1", [D, D], DT2, kind="ExternalInput")
    bm2 = nc.dram_tensor("bm2", [D, D], DT2, kind="ExternalInput")
    h1s = nc.dram_tensor("h1s", [SLICE_PAD, D], DT2)
    h1f = nc.dram_tensor("h1f", [NPAD, D], DT2, addr_space="Shared")
    out = nc.dram_tensor("out", [SLICE_PAD, D], DT2, kind="ExternalOutput")

    bank_col0 = np.concatenate([[0], np.cumsum(nchunks_b * P // 16)]).astype(int)

    # 2048-idx gather batches: per bank, runs of <= 16 chunks
    batches = []
    for b in range(NBANKS):
        c0 = 0
        while c0 < nchunks_b[b]:
            nb_ = min(GATHER_BATCH // P, int(nchunks_b[b]) - c0)
            batches.append((b, c0, nb_))
            c0 += nb_
    pos2batch = {}
    for bi, (b, c0, nchk) in enumerate(batches):
        for jj in range(nchk):
            pos2batch[(b, c0 + jj)] = (bi, jj)
    c2b = {}
    for k in range(NW):
        for b in range(NBANKS):
            for t in range(chunks2[k, b]):
                c2b[(k, b, t)] = pos2batch[(b, chunk_of2[k, b] + t)]
    bank_batches = {b: [bi for bi, (bb, _, _) in enumerate(batches) if bb == b]
                    for b in range(NBANKS)}

    with tile.TileContext(nc) as tc:
        with tc.tile_pool(name="const", bufs=1) as cpool, \
             tc.tile_pool(name="meta", bufs=1) as mpool, \
             tc.tile_pool(name="ms", bufs=2) as mspool, \
             tc.tile_pool(name="oh", bufs=2) as ohpool, \
             tc.tile_pool(name="ohd", bufs=6) as ohdpool, \
             tc.tile_pool(name="gat", bufs=4) as gpool, \
             tc.tile_pool(name="grp", bufs=2) as grpool, \
             tc.tile_pool(name="sm", bufs=3) as smpool, \
             tc.tile_pool(name="st", bufs=2) as stpool, \
             tc.tile_pool(name="pa", bufs=2, space="PSUM") as papool, \
             tc.tile_pool(name="px", bufs=2, space="PSUM") as pxpool, \
             tc.tile_pool(name="po", bufs=2, space="PSUM") as popool:

            ident_f = cpool.tile([P, P], DT)
            make_identity(nc, ident_f[:])
            ident = cpool.tile([P, P], DT2)
            nc.vector.tensor_copy(ident[:], ident_f[:])
            iota_i = cpool.tile([P, P], mybir.dt.int32)
            nc.gpsimd.iota(iota_i[:], pattern=[[1, P]], base=0, channel_multiplier=0)
            iota_f = cpool.tile([P, P], DT2)
            nc.vector.tensor_copy(iota_f[:], iota_i[:])
            # U0: row 0 all ones (for bias matmul)
            u0 = cpool.tile([P, P], DT2)
            nc.vector.memset(u0[:], 0.0)
            nc.vector.memset(u0[0:1, :], 1.0)

            wa = [cpool.tile([P, D], DT2, name=f"wa{l}") for l in range(2)]
            wb = [cpool.tile([P, D], DT2, name=f"wb{l}") for l in range(2)]
            bm = [cpool.tile([P, D], DT2, name=f"bm{l}") for l in range(2)]
            for li, (wat, wbt, bmt) in enumerate(((wa1, wb1, bm1), (wa2, wb2, bm2))):
                nc.sync.dma_start(out=wa[li][:], in_=wat[:, :])
                nc.sync.dma_start(out=wb[li][:], in_=wbt[:, :])
                nc.sync.dma_start(out=bm[li][:], in_=bmt[:, :])

            gidx_sb = mpool.tile([P, idxcols], mybir.dt.int16)
            nc.sync.dma_start(out=gidx_sb[:], in_=gidx[:])
            recv1_sb = mpool.tile([P, max(tot1, 1)], DT)
            nc.sync.dma_start(out=recv1_sb[:], in_=recv1[:])
            wch1_sb = mpool.tile([P, max(tot1, 1)], DT)
            nc.sync.dma_start(out=wch1_sb[:], in_=wch1[:])
            selfw_sb = mpool.tile([P, NW], DT)
            nc.sync.dma_start(out=selfw_sb[:], in_=selfw1[:])

            relu_t = mybir.ActivationFunctionType.Relu
            copy_t = mybir.ActivationFunctionType.Copy

            for layer in range(2):
                xsrc = x0g if layer == 0 else h1s
                ot = ot2
                dst = h1s if layer == 0 else out

                # --- L2: lazy 2048-idx gather batches on async queues 1-3 ---
                gtiles = {}
                bank_next = [0] * NBANKS

                def ensure_batch(bi):
                    b = batches[bi][0]
                    while bi not in gtiles:
                        nb = bank_batches[b][bank_next[b]]
                        bank_next[b] += 1
                        _, c0, nchk = batches[nb]
                        nidx = nchk * P
                        gt = gpool.tile([P, nchk, D], DT2,
                                        tag=f"gq{nb % 3}")
                        col0 = bank_col0[b] + c0 * P // 16
                        nc.gpsimd.dma_gather(
                            gt[:],
                            h1f[b * BROWS:(b + 1) * BROWS, :],
                            gidx_sb[:, col0:col0 + nidx // 16],
                            nidx, nidx, D,
                            single_packet=False,
                            queue_num=1 + (nb % 3),
                        )
                        gtiles[nb] = gt
                    return gtiles[bi]

                # --- stream state: msg pieces (L1) and oh pieces (L2) ---
                oh_piece = [None, -1, -1]   # tile, t0, t1
                ms_piece = [None, -1, -1]

                def oh_slice(t):
                    if not (oh_piece[1] <= t < oh_piece[2]):
                        t0 = t
                        te = min(t0 + LCH, totoh2)
                        pt = ohpool.tile([P, te - t0, D], DT2, tag="ohp")
                        nc.sync.dma_start(out=pt[:], in_=oh2[:, t0:te, :])
                        oh_piece[0], oh_piece[1], oh_piece[2] = pt, t0, te
                    return oh_piece[0][:, t - oh_piece[1], :]

                def ms_slice(t):
                    if not (ms_piece[1] <= t < ms_piece[2]):
                        t0 = t
                        te = min(t0 + LCH, tot1)
                        pt = mspool.tile([P, te - t0, D], DT2, tag="msp")
                        nc.sync.dma_start(out=pt[:], in_=msg1[:, t0:te, :])
                        ms_piece[0], ms_piece[1], ms_piece[2] = pt, t0, te
                    return ms_piece[0][:, t - ms_piece[1], :]

                oh1_piece = [None, -1, -1]

                def oh1h_slice(t):
                    if not (oh1_piece[1] <= t < oh1_piece[2]):
                        t0 = t
                        te = min(t0 + LCH, toth)
                        pt = ohpool.tile([P, te - t0, D], DT2, tag="ohp")
                        nc.sync.dma_start(out=pt[:], in_=oh1h[:, t0:te, :])
                        oh1_piece[0], oh1_piece[1], oh1_piece[2] = pt, t0, te
                    return oh1_piece[0][:, t - oh1_piece[1], :]

                for g in range(NG):
                    nwg = sizes[g]
                    grp = grpool.tile([P, nwg, D], DT2, tag="grp")
                    nc.sync.dma_start(
                        out=grp[:], in_=xsrc[gstart[g]:gstart[g] + nwg * P, :])
                    stage = stpool.tile([P, nwg, D], DT2, tag=f"stg{layer}")

                    for j in range(nwg):
                        k = g * GRPW + j
                        psum = papool.tile([P, P], mybir.dt.float32, space="PSUM")
                        first = True
                        if layer == 0:
                            for t in range(chunks1[k]):
                                mt = ms_slice(t1[k] + t)
                                if t % 2 == 1:
                                    oht_ap = oh1h_slice(hoff[k] + t // 2)
                                else:
                                    oht = ohdpool.tile([P, P], DT2, tag="ohb")
                                    nc.vector.tensor_scalar(
                                        out=oht[:], in0=iota_f[:],
                                        scalar1=recv1_sb[:, t1[k] + t:t1[k] + t + 1],
                                        scalar2=wch1_sb[:, t1[k] + t:t1[k] + t + 1],
                                        op0=mybir.AluOpType.is_equal,
                                        op1=mybir.AluOpType.mult)
                                    oht_ap = oht[:]
                                nc.tensor.matmul(out=psum[:], lhsT=mt, rhs=oht_ap,
                                                 start=first, stop=False)
                                first = False
                            # self: diag(selfw) built from identity
                            oht = ohdpool.tile([P, P], DT2, tag="ohb")
                            nc.vector.tensor_scalar(
                                out=oht[:], in0=ident[:],
                                scalar1=selfw_sb[:, k:k + 1], scalar2=None,
                                op0=mybir.AluOpType.mult)
                            nc.tensor.matmul(out=psum[:], lhsT=grp[:, j, :],
                                             rhs=oht[:], start=first, stop=True)
                        else:
                            u = 0
                            for b in range(NBANKS):
                                for t in range(chunks2[k, b]):
                                    bi, jj = c2b[(k, b, t)]
                                    gt = ensure_batch(bi)
                                    oht = oh_slice(ot[k] + u)
                                    nc.tensor.matmul(
                                        out=psum[:], lhsT=gt[:, jj, :], rhs=oht,
                                        start=first, stop=False)
                                    first = False
                                    u += 1
                            oht = oh_slice(ot[k] + uk[k])
                            nc.tensor.matmul(out=psum[:], lhsT=grp[:, j, :],
                                             rhs=oht, start=first, stop=True)

                        psx = pxpool.tile([P, P], mybir.dt.float32, space="PSUM")
                        nc.tensor.matmul(out=psx[:], lhsT=grp[:, j, :], rhs=ident[:],
                                         start=True, stop=True)

                        summed = smpool.tile([P, P], DT2, tag="summed")
                        nc.vector.tensor_copy(summed[:], psum[:])
                        xt = smpool.tile([P, P], DT2, tag="xt")
                        nc.vector.tensor_copy(xt[:], psx[:])

                        pso = popool.tile([P, P], mybir.dt.float32, space="PSUM")
                        nc.tensor.matmul(out=pso[:], lhsT=u0[:], rhs=bm[layer][:],
                                         start=True, stop=False)
                        nc.tensor.matmul(out=pso[:], lhsT=xt[:], rhs=wa[layer][:],
                                         start=False, stop=False)
                        nc.tensor.matmul(out=pso[:], lhsT=summed[:], rhs=wb[layer][:],
                                         start=False, stop=True)
                        nc.scalar.activation(
                            out=stage[:, j, :], in_=pso[:],
                            func=relu_t if layer == 0 else copy_t)

                    nc.sync.dma_start(
                        out=dst[gstart[g]:gstart[g] + nwg * P, :], in_=stage[:])

                if layer == 0:
                    nc.gpsimd.collective_compute(
                        kind="AllGather",
                        op=mybir.AluOpType.bypass,
                        replica_groups=[list(range(NC))],
                        ins=[h1s[:, :]],
                        outs=[h1f[:, :]],
                    )
    nc.compile()
    return nc


def _prep(gid, senders, receivers, emb_table, W1, b1, W2, b2):
    """Host-side layout computation. Returns (layout, in_maps)."""
    gid = np.asarray(gid)
    s = np.asarray(senders).astype(np.int64)
    r = np.asarray(receivers).astype(np.int64)
    emb = np.asarray(emb_table, dtype=np.float32)
    W1 = np.asarray(W1, np.float32)
    W2 = np.asarray(W2, np.float32)
    b1 = np.asarray(b1, np.float32)
    b2 = np.asarray(b2, np.float32)

    x0 = emb[gid]
    x_bf = x0.astype(BF16)

    ds = (1 + np.bincount(s, minlength=N)).astype(np.float64)
    dr = (1 + np.bincount(r, minlength=N)).astype(np.float64)
    wch = (1.0 / np.sqrt(ds[s] * dr[r] ** 3)).astype(np.float32)
    selfw = (1.0 / np.sqrt(ds * dr ** 3)).astype(np.float32)

    prow = _perm()                      # local l -> permuted row

    core_of = r // SLICE
    rloc = r % SLICE
    k_all = rloc // P
    rcol_all = rloc % P
    g_all = (s // SLICE) * SLICE_PAD + prow[s % SLICE]
    bank_all = g_all // BROWS
    brow_all = (g_all % BROWS).astype(np.int16)

    # ---- per-core per-window counts -> shared chunk layout ----
    cnt1 = np.zeros((NC, NW), np.int64)
    np.add.at(cnt1, (core_of, k_all), 1)
    chunks1 = np.ceil(cnt1.max(axis=0) / P).astype(np.int64)   # [NW]
    cnt2 = np.zeros((NC, NW, NBANKS), np.int64)
    np.add.at(cnt2, (core_of, k_all, bank_all), 1)
    chunks2 = np.ceil(cnt2.max(axis=0) / P).astype(np.int64)   # [NW, NBANKS]

    tot1 = int(chunks1.sum())
    totoh1 = int((chunks1 + 1).sum())
    uk = chunks2.sum(axis=1)
    totoh2 = int((uk + 1).sum())

    # bank chunk positions: bank b chunks ordered by (k, t)
    chunk_of2 = np.zeros((NW, NBANKS), np.int64)
    nchunks_b = np.zeros(NBANKS, np.int64)
    for b in range(NBANKS):
        pos = 0
        for k in range(NW):
            chunk_of2[k, b] = pos
            pos += chunks2[k, b]
        nchunks_b[b] = pos

    idxcols = int(nchunks_b.sum()) * P // 16

    # hybrid L1 one-hots: odd chunks DMA'd from a stream, even chunks DVE-built
    nodd = chunks1 // 2
    hoff = np.concatenate([[0], np.cumsum(nodd)])
    toth = int(nodd.sum())

    ot1 = np.concatenate([[0], np.cumsum(chunks1 + 1)])
    t1o = np.concatenate([[0], np.cumsum(chunks1)])
    ot2 = np.concatenate([[0], np.cumsum(uk + 1)])
    # within-window oh position offset for (b): cumsum of chunks2 row
    boff = np.zeros((NW, NBANKS), np.int64)
    for k in range(NW):
        acc = 0
        for b in range(NBANKS):
            boff[k, b] = acc
            acc += chunks2[k, b]

    Wa1 = W1[:D].astype(BF16)
    Wb1 = W1[D:].astype(BF16)
    Wa2 = W2[:D].astype(BF16)
    Wb2 = W2[D:].astype(BF16)
    bm1 = np.zeros((D, D), BF16)
    bm1[0, :] = b1.astype(BF16)
    bm2 = np.zeros((D, D), BF16)
    bm2[0, :] = b2.astype(BF16)

    # selfw values per (window, p) for each core, 0 on pad nodes
    in_maps = []
    for c in range(NC):
        m = core_of == c
        km = k_all[m]
        rcolm = rcol_all[m]
        sm = s[m]
        wchm = wch[m]
        bankm = bank_all[m]
        browm = brow_all[m]

        # ---- layer 1 slots: sort by window ----
        o1 = np.argsort(km, kind="stable")
        k1s = km[o1]
        # within-window running index
        ww = np.ones(len(k1s), np.int64)
        first = np.where(np.diff(k1s, prepend=-1) != 0)[0]
        run = np.arange(len(k1s)) - first[np.searchsorted(first, np.arange(len(k1s)), side="right") - 1]
        t_1 = run // P
        p_1 = run % P
        tglob1 = t1o[k1s] + t_1
        ohglob1 = ot1[k1s] + t_1

        msg1 = np.zeros((P, max(tot1, 1), D), BF16)
        msg1[p_1, tglob1, :] = x_bf[sm[o1]]
        recv1 = np.full((P, max(tot1, 1)), -1000.0, np.float32)
        recv1[p_1, tglob1] = rcolm[o1]
        wch1 = np.ones((P, max(tot1, 1)), np.float32)
        wch1[p_1, tglob1] = wchm[o1]
        # odd chunks also packed into the oh1h DMA stream
        odd = (t_1 % 2) == 1
        hglob = hoff[k1s] + t_1 // 2
        oh1h = np.zeros((P, max(toth, 1), D), BF16)
        oh1h[p_1[odd], hglob[odd], rcolm[o1][odd]] = wchm[o1][odd].astype(BF16)
        # self-loop weights per (p, window), 0 on pad nodes
        node_l = np.arange(SLICE_PAD)
        valid = node_l < SLICE
        kk = node_l // P
        pp = node_l % P
        sw = np.zeros(SLICE_PAD, np.float32)
        sw[valid] = selfw[c * SLICE + node_l[valid]]
        selfw1 = np.zeros((P, NW), np.float32)
        selfw1[pp, kk] = sw

        # ---- layer 2 slots: sort by (window, bank) ----
        o2 = np.lexsort((bankm, km))
        k2s = km[o2]
        b2s = bankm[o2]
        gid2 = k2s * NBANKS + b2s
        first2 = np.where(np.diff(gid2, prepend=-1) != 0)[0]
        run2 = np.arange(len(gid2)) - first2[np.searchsorted(first2, np.arange(len(gid2)), side="right") - 1]
        t_2 = run2 // P
        p_2 = run2 % P
        ohglob2 = ot2[k2s] + boff[k2s, b2s] + t_2
        oh2 = np.zeros((P, totoh2, D), BF16)
        oh2[p_2, ohglob2, rcolm[o2]] = wchm[o2].astype(BF16)
        oh2[pp, ot2[kk] + uk[kk], pp] = sw.astype(BF16)

        # gather idx per bank. Pads at each (k,b)-region tail become -1
        # (skipped by the gather ucode), except the partial trailing
        # 16-group which uses safe idx 0 (killed by zero one-hot cols).
        idx16 = []
        cpos2 = chunk_of2[k2s, b2s] + t_2
        cnt_cb = cnt2[c]                       # [NW, NBANKS] this core's counts
        for b in range(NBANKS):
            mb = b2s == b
            st = np.zeros(int(nchunks_b[b]) * P, np.int16)
            if PAD_NEG:
                for k in range(NW):
                    nchk = chunks2[k, b]
                    if nchk == 0:
                        continue
                    lo = chunk_of2[k, b] * P
                    valid16 = -(-int(cnt_cb[k, b]) // 16) * 16
                    st[lo + valid16: lo + nchk * P] = -1
            st[cpos2[mb] * P + p_2[mb]] = browm[o2][mb]
            a = st.reshape(len(st) // 16, 16).T.copy()
            idx16.append(np.tile(a, (8, 1)))
        gidx_np = np.concatenate(idx16, axis=1) if idxcols else np.zeros((P, 0), np.int16)

        # x0 permuted table
        x0g = np.zeros((SLICE_PAD, D), BF16)
        x0g[prow[node_l[valid]]] = x_bf[c * SLICE + node_l[valid]]

        in_maps.append({
            "x0g": x0g, "msg1": msg1, "recv1": recv1, "wch1": wch1,
            "selfw1": selfw1, "oh2": oh2, "oh1h": oh1h, "gidx": gidx_np,
            "wa1": Wa1, "wb1": Wb1, "wa2": Wa2, "wb2": Wb2,
            "bm1": bm1, "bm2": bm2,
        })

    layout = dict(chunks1=chunks1, chunks2=chunks2, nchunks_b=nchunks_b,
                  chunk_of2=chunk_of2, idxcols=idxcols,
                  tot1=tot1, totoh2=totoh2, hoff=hoff, toth=toth)
    return layout, in_maps, prow


def kernel(gid, senders, receivers, is_training, emb_table, W1, b1, W2, b2):
    global _last_results
    from concourse.bass_utils import run_bass_kernel_spmd

    layout, in_maps, prow = _prep(gid, senders, receivers, emb_table,
                                  W1, b1, W2, b2)
    nc = _build_program(layout["chunks1"], layout["chunks2"],
                        layout["nchunks_b"], layout["chunk_of2"],
                        layout["idxcols"], layout["tot1"], layout["totoh2"],
                        layout["hoff"], layout["toth"])

    res = run_bass_kernel_spmd(nc, in_maps, core_ids=list(range(NC)))
    _last_results = res

    out = np.empty((N, D), np.float32)
    l = np.arange(SLICE)
    for c in range(NC):
        out[c * SLICE:(c + 1) * SLICE] = \
            res.results[c]["out"][prow[l]].astype(np.float32)
    return out


# revision 27
# speedup vs baseline: 1.1379x; 1.0089x over previous
"""Trainium2 Bass kernel for nn_NodeEncoder (2-layer SAGEConv GNN).

Self-contained: takes FULL inputs, shards receivers across 8 NeuronCores,
runs a Bass/Tile kernel via run_bass_kernel_spmd, returns the FULL output.

Math per layer (SAGEConv, degree_norm=True, self loops):
  x_upd[r] = sum_{e: recv=r} w_e * x[s_e] + selfw_r * x[r],
     w_e = (ds[s_e] * dr[r_e]^3)^-1/2, selfw_n = (ds[n]*dr[n]^3)^-1/2
  out = x @ Wa + x_upd @ Wb + b   (+relu after layer 1)

Device mapping (per core, receivers sharded, 98 windows of 128 nodes):
  - psum_agg[f, r]  = sum_chunks  msg_chunk[e,f].T @ onehot_chunk[e,r]
      onehot[e, r] = w_e * (recv_e == r)  -- HOST-precomputed, DMA streamed
      L1 msgs: HOST-pregathered sequential stream;  L2: dma_gather from
      the AllGathered h1 table on async SWDGE queues 1-3
      self loop: lhsT = row tile [n,f], rhs = diag(selfw) (in onehot stream)
  - psum_xt[f, n]   = rowtile[n,f].T @ I       (transpose via PE)
  - psum_out[n, fo] = U0.T@Bmat (bias) + xt[f,n].T@Wa + summed[f,n].T@Wb
  - node tables (x0, h1, out) use a permuted "group" layout so loads and
    stores of 8-window groups are single contiguous DMAs
"""

import numpy as np
import ml_dtypes

BF16 = ml_dtypes.bfloat16
N = 100000
E = 600000
D = 128
NC = 8
P = 128
SLICE = N // NC                 # 12500
NW = (SLICE + P - 1) // P       # 98 windows
SLICE_PAD = NW * P              # 12544
GRPW = 8                        # windows per group
NG = (NW + GRPW - 1) // GRPW    # 13 groups (12x8 + 1x2)
NPAD = SLICE_PAD * NC           # 100352
NBANKS = 4
BROWS = NPAD // NBANKS          # 25088 (< 32768, int16-safe)
GATHER_BATCH = 2048             # idxs per dma_gather instruction
PAD_NEG = False                 # -1 idx pads (skipped) vs safe 0 pads
LCH = 24                        # stream-load piece size (chunks)

_last_results = None


def _grp_sizes():
    return [min(GRPW, NW - g * GRPW) for g in range(NG)]


def _perm():
    """prow[l] = permuted row of local node l (group layout)."""
    l = np.arange(SLICE_PAD)
    k = l // P
    p = l % P
    g = k // GRPW
    j = k - g * GRPW
    base = np.minimum(g, 12) * (GRPW * P) * 1  # recompute below correctly
    # base offset of group g = sum of sizes of previous groups * P
    sizes = _grp_sizes()
    starts = np.concatenate([[0], np.cumsum([s * P for s in sizes])])
    nwg = np.array(sizes)[g]
    return starts[g] + p * nwg + j


def _build_program(chunks1, chunks2, nchunks_b, chunk_of2,
                   idxcols, tot1, totoh2, hoff, toth):
    import concourse.bacc as bacc
    import concourse.mybir as mybir
    import concourse.tile as tile
    from concourse.masks import make_identity

    DT = mybir.dt.float32
    DT2 = mybir.dt.bfloat16
    sizes = _grp_sizes()
    gstart = np.concatenate([[0], np.cumsum([s * P for s in sizes])])

    # window -> (oh-chunk offset, n edge chunks) per layer
    t1 = np.concatenate([[0], np.cumsum(chunks1)])
    uk = chunks2.sum(axis=1)
    ot2 = np.concatenate([[0], np.cumsum(uk + 1)])

    nc = bacc.Bacc("TRN2", target_bir_lowering=False, num_swdge_queues=4)

    x0g = nc.dram_tensor("x0g", [SLICE_PAD, D], DT2, kind="ExternalInput")
    msg1 = nc.dram_tensor("msg1", [P, max(tot1, 1), D], DT2, kind="ExternalInput")
    recv1 = nc.dram_tensor("recv1", [P, max(tot1, 1)], DT, kind="ExternalInput")
    wch1 = nc.dram_tensor("wch1", [P, max(tot1, 1)], DT, kind="ExternalInput")
    selfw1 = nc.dram_tensor("selfw1", [P, NW], DT, kind="ExternalInput")
    oh2 = nc.dram_tensor("oh2", [P, totoh2, D], DT2, kind="ExternalInput")
    oh1h = nc.dram_tensor("oh1h", [P, max(toth, 1), D], DT2, kind="ExternalInput")
    gidx = nc.dram_tensor("gidx", [P, idxcols], mybir.dt.int16, kind="ExternalInput")
    wa1 = nc.dram_tensor("wa1", [D, D], DT2, kind="ExternalInput")
    wb1 = nc.dram_tensor("wb1", [D, D], DT2, kind="ExternalInput")
    wa2 = nc.dram_tensor("wa2", [D, D], DT2, kind="ExternalInput")
    wb2 = nc.dram_tensor("wb2", [D, D], DT2, kind="ExternalInput")
    bm1 = nc.dram_tensor("bm1", [D, D], DT2, kind="ExternalInput")
    bm2 = nc.dram_tensor("bm2", [D, D], DT2, kind="ExternalInput")
    h1s = nc.dram_tensor("h1s", [SLICE_PAD, D], DT2)
    h1f = nc.dram_tensor("h1f", [NPAD, D], DT2, addr_space="Shared")
    out = nc.dram_tensor("out", [SLICE_PAD, D], DT2, kind="ExternalOutput")

    bank_col0 = np.concatenate([[0], np.cumsum(nchunks_b * P // 16)]).astype(int)

    # 2048-idx gather batches: per bank, runs of <= 16 chunks
    batches = []
    for b in range(NBANKS):
        c0 = 0
        while c0 < nchunks_b[b]:
            nb_ = min(GATHER_BATCH // P, int(nchunks_b[b]) - c0)
            batches.append((b, c0, nb_))
            c0 += nb_
    pos2batch = {}
    for bi, (b, c0, nchk) in enumerate(batches):
        for jj in range(nchk):
            pos2batch[(b, c0 + jj)] = (bi, jj)
    c2b = {}
    for k in range(NW):
        for b in range(NBANKS):
            for t in range(chunks2[k, b]):
                c2b[(k, b, t)] = pos2batch[(b, chunk_of2[k, b] + t)]
    bank_batches = {b: [bi for bi, (bb, _, _) in enumerate(batches) if bb == b]
                    for b in range(NBANKS)}

    with tile.TileContext(nc) as tc:
        with tc.tile_pool(name="const", bufs=1) as cpool, \
             tc.tile_pool(name="meta", bufs=1) as mpool, \
             tc.tile_pool(name="ms", bufs=2) as mspool, \
             tc.tile_pool(name="oh", bufs=2) as ohpool, \
             tc.tile_pool(name="ohd", bufs=6) as ohdpool, \
             tc.tile_pool(name="gat", bufs=3) as gpool, \
             tc.tile_pool(name="grp", bufs=2) as grpool, \
             tc.tile_pool(name="sm", bufs=3) as smpool, \
             tc.tile_pool(name="st", bufs=2) as stpool, \
             tc.tile_pool(name="pa", bufs=2, space="PSUM") as papool, \
             tc.tile_pool(name="px", bufs=2, space="PSUM") as pxpool, \
             tc.tile_pool(name="po", bufs=2, space="PSUM") as popool:

            ident_f = cpool.tile([P, P], DT)
            make_identity(nc, ident_f[:])
            ident = cpool.tile([P, P], DT2)
            nc.vector.tensor_copy(ident[:], ident_f[:])
            iota_i = cpool.tile([P, P], mybir.dt.int32)
            nc.gpsimd.iota(iota_i[:], pattern=[[1, P]], base=0, channel_multiplier=0)
            iota_f = cpool.tile([P, P], DT2)
            nc.vector.tensor_copy(iota_f[:], iota_i[:])
            # U0: row 0 all ones (for bias matmul)
            u0 = cpool.tile([P, P], DT2)
            nc.vector.memset(u0[:], 0.0)
            nc.vector.memset(u0[0:1, :], 1.0)

            wa = [cpool.tile([P, D], DT2, name=f"wa{l}") for l in range(2)]
            wb = [cpool.tile([P, D], DT2, name=f"wb{l}") for l in range(2)]
            bm = [cpool.tile([P, D], DT2, name=f"bm{l}") for l in range(2)]
            for li, (wat, wbt, bmt) in enumerate(((wa1, wb1, bm1), (wa2, wb2, bm2))):
                nc.sync.dma_start(out=wa[li][:], in_=wat[:, :])
                nc.sync.dma_start(out=wb[li][:], in_=wbt[:, :])
                nc.sync.dma_start(out=bm[li][:], in_=bmt[:, :])

            gidx_sb = mpool.tile([P, idxcols], mybir.dt.int16)
            nc.sync.dma_start(out=gidx_sb[:], in_=gidx[:])
            recv1_sb = mpool.tile([P, max(tot1, 1)], DT)
            nc.sync.dma_start(out=recv1_sb[:], in_=recv1[:])
            wch1_sb = mpool.tile([P, max(tot1, 1)], DT)
            nc.sync.dma_start(out=wch1_sb[:], in_=wch1[:])
            selfw_sb = mpool.tile([P, NW], DT)
            nc.sync.dma_start(out=selfw_sb[:], in_=selfw1[:])

            relu_t = mybir.ActivationFunctionType.Relu
            copy_t = mybir.ActivationFunctionType.Copy

            for layer in range(2):
                xsrc = x0g if layer == 0 else h1s
                ot = ot2
                dst = h1s if layer == 0 else out

                # --- L2: lazy 2048-idx gather batches on async queues 1-3 ---
                gtiles = {}
                bank_next = [0] * NBANKS

                QMAP = (1, 2, 3, 0)   # q0 last per round: its sync descgen
                                      # only blocks the otherwise-idle Pool

                def ensure_batch(bi):
                    b = batches[bi][0]
                    while bi not in gtiles:
                        nb = bank_batches[b][bank_next[b]]
                        bank_next[b] += 1
                        _, c0, nchk = batches[nb]
                        nidx = nchk * P
                        q = QMAP[nb % 4]
                        gt = gpool.tile([P, nchk, D], DT2, tag=f"gq{q}")
                        col0 = bank_col0[b] + c0 * P // 16
                        nc.gpsimd.dma_gather(
                            gt[:],
                            h1f[b * BROWS:(b + 1) * BROWS, :],
                            gidx_sb[:, col0:col0 + nidx // 16],
                            nidx, nidx, D,
                            single_packet=False,
                            queue_num=q,
                        )
                        gtiles[nb] = gt
                    return gtiles[bi]

                # --- stream state: msg pieces (L1) and oh pieces (L2) ---
                oh_piece = [None, -1, -1]   # tile, t0, t1
                ms_piece = [None, -1, -1]

                def oh_slice(t):
                    if not (oh_piece[1] <= t < oh_piece[2]):
                        t0 = t
                        te = min(t0 + LCH, totoh2)
                        pt = ohpool.tile([P, te - t0, D], DT2, tag="ohp")
                        nc.sync.dma_start(out=pt[:], in_=oh2[:, t0:te, :])
                        oh_piece[0], oh_piece[1], oh_piece[2] = pt, t0, te
                    return oh_piece[0][:, t - oh_piece[1], :]

                def ms_slice(t):
                    if not (ms_piece[1] <= t < ms_piece[2]):
                        t0 = t
                        te = min(t0 + LCH, tot1)
                        pt = mspool.tile([P, te - t0, D], DT2, tag="msp")
                        nc.sync.dma_start(out=pt[:], in_=msg1[:, t0:te, :])
                        ms_piece[0], ms_piece[1], ms_piece[2] = pt, t0, te
                    return ms_piece[0][:, t - ms_piece[1], :]

                oh1_piece = [None, -1, -1]

                def oh1h_slice(t):
                    if not (oh1_piece[1] <= t < oh1_piece[2]):
                        t0 = t
                        te = min(t0 + LCH, toth)
                        pt = ohpool.tile([P, te - t0, D], DT2, tag="ohp")
                        nc.sync.dma_start(out=pt[:], in_=oh1h[:, t0:te, :])
                        oh1_piece[0], oh1_piece[1], oh1_piece[2] = pt, t0, te
                    return oh1_piece[0][:, t - oh1_piece[1], :]

                for g in range(NG):
                    nwg = sizes[g]
                    grp = grpool.tile([P, nwg, D], DT2, tag="grp")
                    nc.sync.dma_start(
                        out=grp[:], in_=xsrc[gstart[g]:gstart[g] + nwg * P, :])
                    stage = stpool.tile([P, nwg, D], DT2, tag=f"stg{layer}")

                    for j in range(nwg):
                        k = g * GRPW + j
                        psum = papool.tile([P, P], mybir.dt.float32, space="PSUM")
                        first = True
                        if layer == 0:
                            for t in range(chunks1[k]):
                                mt = ms_slice(t1[k] + t)
                                if t % 2 == 1:
                                    oht_ap = oh1h_slice(hoff[k] + t // 2)
                                else:
                                    oht = ohdpool.tile([P, P], DT2, tag="ohb")
                                    nc.vector.tensor_scalar(
                                        out=oht[:], in0=iota_f[:],
                                        scalar1=recv1_sb[:, t1[k] + t:t1[k] + t + 1],
                                        scalar2=wch1_sb[:, t1[k] + t:t1[k] + t + 1],
                                        op0=mybir.AluOpType.is_equal,
                                        op1=mybir.AluOpType.mult)
                                    oht_ap = oht[:]
                                nc.tensor.matmul(out=psum[:], lhsT=mt, rhs=oht_ap,
                                                 start=first, stop=False)
                                first = False
                            # self: diag(selfw) built from identity
                            oht = ohdpool.tile([P, P], DT2, tag="ohb")
                            nc.vector.tensor_scalar(
                                out=oht[:], in0=ident[:],
                                scalar1=selfw_sb[:, k:k + 1], scalar2=None,
                                op0=mybir.AluOpType.mult)
                            nc.tensor.matmul(out=psum[:], lhsT=grp[:, j, :],
                                             rhs=oht[:], start=first, stop=True)
                        else:
                            u = 0
                            for b in range(NBANKS):
                                for t in range(chunks2[k, b]):
                                    bi, jj = c2b[(k, b, t)]
                                    gt = ensure_batch(bi)
                                    oht = oh_slice(ot[k] + u)
                                    nc.tensor.matmul(
                                        out=psum[:], lhsT=gt[:, jj, :], rhs=oht,
                                        start=first, stop=False)
                                    first = False
                                    u += 1
                            oht = oh_slice(ot[k] + uk[k])
                            nc.tensor.matmul(out=psum[:], lhsT=grp[:, j, :],
                                             rhs=oht, start=first, stop=True)

                        psx = pxpool.tile([P, P], mybir.dt.float32, space="PSUM")
                        nc.tensor.matmul(out=psx[:], lhsT=grp[:, j, :], rhs=ident[:],
                                         start=True, stop=True)

                        summed = smpool.tile([P, P], DT2, tag="summed")
                        nc.vector.tensor_copy(summed[:], psum[:])
                        xt = smpool.tile([P, P], DT2, tag="xt")
                        nc.vector.tensor_copy(xt[:], psx[:])

                        pso = popool.tile([P, P], mybir.dt.float32, space="PSUM")
                        nc.tensor.matmul(out=pso[:], lhsT=u0[:], rhs=bm[layer][:],
                                         start=True, stop=False)
                        nc.tensor.matmul(out=pso[:], lhsT=xt[:], rhs=wa[layer][:],
                                         start=False, stop=False)
                        nc.tensor.matmul(out=pso[:], lhsT=summed[:], rhs=wb[layer][:],
                                         start=False, stop=True)
                        nc.scalar.activation(
                            out=stage[:, j, :], in_=pso[:],
                            func=relu_t if layer == 0 else copy_t)

                    nc.sync.dma_start(
                        out=dst[gstart[g]:gstart[g] + nwg * P, :], in_=stage[:])

                if layer == 0:
                    nc.gpsimd.collective_compute(
                        kind="AllGather",
                        op=mybir.AluOpType.bypass,
                        replica_groups=[list(range(NC))],
                        ins=[h1s[:, :]],
                        outs=[h1f[:, :]],
                    )
    nc.compile()
    return nc


def _prep(gid, senders, receivers, emb_table, W1, b1, W2, b2):
    """Host-side layout computation. Returns (layout, in_maps)."""
    gid = np.asarray(gid)
    s = np.asarray(senders).astype(np.int64)
    r = np.asarray(receivers).astype(np.int64)
    emb = np.asarray(emb_table, dtype=np.float32)
    W1 = np.asarray(W1, np.float32)
    W2 = np.asarray(W2, np.float32)
    b1 = np.asarray(b1, np.float32)
    b2 = np.asarray(b2, np.float32)

    x0 = emb[gid]
    x_bf = x0.astype(BF16)

    ds = (1 + np.bincount(s, minlength=N)).astype(np.float64)
    dr = (1 + np.bincount(r, minlength=N)).astype(np.float64)
    wch = (1.0 / np.sqrt(ds[s] * dr[r] ** 3)).astype(np.float32)
    selfw = (1.0 / np.sqrt(ds * dr ** 3)).astype(np.float32)

    prow = _perm()                      # local l -> permuted row

    core_of = r // SLICE
    rloc = r % SLICE
    k_all = rloc // P
    rcol_all = rloc % P
    g_all = (s // SLICE) * SLICE_PAD + prow[s % SLICE]
    bank_all = g_all // BROWS
    brow_all = (g_all % BROWS).astype(np.int16)

    # ---- per-core per-window counts -> shared chunk layout ----
    cnt1 = np.zeros((NC, NW), np.int64)
    np.add.at(cnt1, (core_of, k_all), 1)
    chunks1 = np.ceil(cnt1.max(axis=0) / P).astype(np.int64)   # [NW]
    cnt2 = np.zeros((NC, NW, NBANKS), np.int64)
    np.add.at(cnt2, (core_of, k_all, bank_all), 1)
    chunks2 = np.ceil(cnt2.max(axis=0) / P).astype(np.int64)   # [NW, NBANKS]

    tot1 = int(chunks1.sum())
    totoh1 = int((chunks1 + 1).sum())
    uk = chunks2.sum(axis=1)
    totoh2 = int((uk + 1).sum())

    # bank chunk positions: bank b chunks ordered by (k, t)
    chunk_of2 = np.zeros((NW, NBANKS), np.int64)
    nchunks_b = np.zeros(NBANKS, np.int64)
    for b in range(NBANKS):
        pos = 0
        for k in range(NW):
            chunk_of2[k, b] = pos
            pos += chunks2[k, b]
        nchunks_b[b] = pos

    idxcols = int(nchunks_b.sum()) * P // 16

    # hybrid L1 one-hots: odd chunks DMA'd from a stream, even chunks DVE-built
    nodd = chunks1 // 2
    hoff = np.concatenate([[0], np.cumsum(nodd)])
    toth = int(nodd.sum())

    ot1 = np.concatenate([[0], np.cumsum(chunks1 + 1)])
    t1o = np.concatenate([[0], np.cumsum(chunks1)])
    ot2 = np.concatenate([[0], np.cumsum(uk + 1)])
    # within-window oh position offset for (b): cumsum of chunks2 row
    boff = np.zeros((NW, NBANKS), np.int64)
    for k in range(NW):
        acc = 0
        for b in range(NBANKS):
            boff[k, b] = acc
            acc += chunks2[k, b]

    Wa1 = W1[:D].astype(BF16)
    Wb1 = W1[D:].astype(BF16)
    Wa2 = W2[:D].astype(BF16)
    Wb2 = W2[D:].astype(BF16)
    bm1 = np.zeros((D, D), BF16)
    bm1[0, :] = b1.astype(BF16)
    bm2 = np.zeros((D, D), BF16)
    bm2[0, :] = b2.astype(BF16)

    # selfw values per (window, p) for each core, 0 on pad nodes
    in_maps = []
    for c in range(NC):
        m = core_of == c
        km = k_all[m]
        rcolm = rcol_all[m]
        sm = s[m]
        wchm = wch[m]
        bankm = bank_all[m]
        browm = brow_all[m]

        # ---- layer 1 slots: sort by window ----
        o1 = np.argsort(km, kind="stable")
        k1s = km[o1]
        # within-window running index
        ww = np.ones(len(k1s), np.int64)
        first = np.where(np.diff(k1s, prepend=-1) != 0)[0]
        run = np.arange(len(k1s)) - first[np.searchsorted(first, np.arange(len(k1s)), side="right") - 1]
        t_1 = run // P
        p_1 = run % P
        tglob1 = t1o[k1s] + t_1
        ohglob1 = ot1[k1s] + t_1

        msg1 = np.zeros((P, max(tot1, 1), D), BF16)
        msg1[p_1, tglob1, :] = x_bf[sm[o1]]
        recv1 = np.full((P, max(tot1, 1)), -1000.0, np.float32)
        recv1[p_1, tglob1] = rcolm[o1]
        wch1 = np.ones((P, max(tot1, 1)), np.float32)
        wch1[p_1, tglob1] = wchm[o1]
        # odd chunks also packed into the oh1h DMA stream
        odd = (t_1 % 2) == 1
        hglob = hoff[k1s] + t_1 // 2
        oh1h = np.zeros((P, max(toth, 1), D), BF16)
        oh1h[p_1[odd], hglob[odd], rcolm[o1][odd]] = wchm[o1][odd].astype(BF16)
        # self-loop weights per (p, window), 0 on pad nodes
        node_l = np.arange(SLICE_PAD)
        valid = node_l < SLICE
        kk = node_l // P
        pp = node_l % P
        sw = np.zeros(SLICE_PAD, np.float32)
        sw[valid] = selfw[c * SLICE + node_l[valid]]
        selfw1 = np.zeros((P, NW), np.float32)
        selfw1[pp, kk] = sw

        # ---- layer 2 slots: sort by (window, bank) ----
        o2 = np.lexsort((bankm, km))
        k2s = km[o2]
        b2s = bankm[o2]
        gid2 = k2s * NBANKS + b2s
        first2 = np.where(np.diff(gid2, prepend=-1) != 0)[0]
        run2 = np.arange(len(gid2)) - first2[np.searchsorted(first2, np.arange(len(gid2)), side="right") - 1]
        t_2 = run2 // P
        p_2 = run2 % P
        ohglob2 = ot2[k2s] + boff[k2s, b2s] + t_2
        oh2 = np.zeros((P, totoh2, D), BF16)
        oh2[p_2, ohglob2, rcolm[o2]] = wchm[o2].astype(BF16)
        oh2[pp, ot2[kk] + uk[kk], pp] = sw.astype(BF16)

        # gather idx per bank. Pads at each (k,b)-region tail become -1
        # (skipped by the gather ucode), except the partial trailing
        # 16-group which uses safe idx 0 (killed by zero one-hot cols).
        idx16 = []
        cpos2 = chunk_of2[k2s, b2s] + t_2
        cnt_cb = cnt2[c]                       # [NW, NBANKS] this core's counts
        for b in range(NBANKS):
            mb = b2s == b
            st = np.zeros(int(nchunks_b[b]) * P, np.int16)
            if PAD_NEG:
                for k in range(NW):
                    nchk = chunks2[k, b]
                    if nchk == 0:
                        continue
                    lo = chunk_of2[k, b] * P
                    valid16 = -(-int(cnt_cb[k, b]) // 16) * 16
                    st[lo + valid16: lo + nchk * P] = -1
            st[cpos2[mb] * P + p_2[mb]] = browm[o2][mb]
            a = st.reshape(len(st) // 16, 16).T.copy()
            idx16.append(np.tile(a, (8, 1)))
        gidx_np = np.concatenate(idx16, axis=1) if idxcols else np.zeros((P, 0), np.int16)

        # x0 permuted table
        x0g = np.zeros((SLICE_PAD, D), BF16)
        x0g[prow[node_l[valid]]] = x_bf[c * SLICE + node_l[valid]]

        in_maps.append({
            "x0g": x0g, "msg1": msg1, "recv1": recv1, "wch1": wch1,
            "selfw1": selfw1, "oh2": oh2, "oh1h": oh1h, "gidx": gidx_np,
            "wa1": Wa1, "wb1": Wb1, "wa2": Wa2, "wb2": Wb2,
            "bm1": bm1, "bm2": bm2,
        })

    layout = dict(chunks1=chunks1, chunks2=chunks2, nchunks_b=nchunks_b,
                  chunk_of2=chunk_of2, idxcols=idxcols,
                  tot1=tot1, totoh2=totoh2, hoff=hoff, toth=toth)
    return layout, in_maps, prow


def kernel(gid, senders, receivers, is_training, emb_table, W1, b1, W2, b2):
    global _last_results
    from concourse.bass_utils import run_bass_kernel_spmd

    layout, in_maps, prow = _prep(gid, senders, receivers, emb_table,
                                  W1, b1, W2, b2)
    nc = _build_program(layout["chunks1"], layout["chunks2"],
                        layout["nchunks_b"], layout["chunk_of2"],
                        layout["idxcols"], layout["tot1"], layout["totoh2"],
                        layout["hoff"], layout["toth"])

    res = run_bass_kernel_spmd(nc, in_maps, core_ids=list(range(NC)))
    _last_results = res

    out = np.empty((N, D), np.float32)
    l = np.arange(SLICE)
    for c in range(NC):
        out[c * SLICE:(c + 1) * SLICE] = \
            res.results[c]["out"][prow[l]].astype(np.float32)
    return out
